# revision 1
# baseline (speedup 1.0000x reference)
"""LDAM hinge loss on 8 Trainium2 NeuronCores (Bass/Tile, data-parallel).

Reference math (per sample i, logits z0,z1, target t in {0,1}):
    d    = z1 - z0
    x    = (1-2t)*d + (t==0 ? D0 : D1)      # D0,D1 ~ 2-4e-6
    loss = sum_i softplus(x_i)              # softplus(x) = log(1+exp(x))

Device formulation (error < 4e-6 relative, dominated by fp32 anyway):
    softplus(-d+D1) = softplus(d-D1) - (d-D1), and since D0,D1 differ by
    ~6e-6 both branches evaluate softplus at w = d + (D0-D1)/2:
        loss ~= sum_i softplus(w_i) - sum_i t_i*(w_i - (D0+D1)/2)
    Term A is a pure scalar-engine chain: u = exp(w); ln(u+1) with fused
    per-partition accumulation (accum_out). Term B is one DVE
    scalar_tensor_tensor with accum_out. Only 2 DVE + 2 ACT ops per tile,
    safely under the HBM DMA roofline (the kernel streams 16 B/sample:
    8 B logit pair + 8 B int64 target, read as int32 low words, stride 2).

Host side: shard N samples contiguously across 8 cores, run SPMD, sum the
8 x [128, NT] partial grids in float64, return float32 scalar.
"""
import sys

sys.path.insert(0, "/opt/trn_rl_repo")

import numpy as np
import concourse.bacc as bacc
import concourse.mybir as mybir
from concourse.tile import TileContext
from concourse.bass_utils import run_bass_kernel_spmd

N = 4194304
N_CORES = 8
NP = N // N_CORES            # samples per core
P = 128
FD_TOTAL = (NP * 2) // P     # f32 elements per partition per core (8192)
F = 4096                     # max x-tile free dim (f32 elems); 2 MiB per x tile
# Shrinking tile schedule (sums to FD_TOTAL=8192): big tiles up front keep
# DMA at full bandwidth; small final tiles cut the post-last-byte serial
# DVE->ACT tail of a single launch from ~7us to ~2us.
TILE_SCHEDULE = [4096, 2048, 1024, 512, 512]
IO_BUFS = 2
MID_BUFS = 4
T_DMA_ENGINE = "scalar"      # issue target DMA from the ACT HWDGE ring
                             # (A/B-measured faster than both on nc.sync)

D0 = 0.5 / 30000.0 / 4.0     # delta for class 0  (C / (w0*n) / 4)
D1 = 0.5 / 70000.0 / 4.0     # delta for class 1

TRACE = False                # set by test harness to collect HW exec time
LAST = None                  # last BassKernelResults (for profiling)

_programs = {}


def _build(t_is_i64: bool, reps: int = 1, f: int = F,
           io_bufs: int = IO_BUFS, mid_bufs: int = MID_BUFS,
           mode: str = "full", t_dma_engine: str = T_DMA_ENGINE,
           rep_barrier: bool = False):
    """reps>1 repeats the whole per-core pipeline in the instruction stream
    (same data, same SBUF slots) — used only for timing-slope measurement.
    mode="dma" drops all compute (DMA floor ablation); t_dma_engine="scalar"
    issues the target DMA from the ACT HWDGE ring instead of SP;
    rep_barrier adds a strict all-engine scheduling barrier per rep."""
    f32 = mybir.dt.float32
    i32 = mybir.dt.int32
    Alu = mybir.AluOpType
    Act = mybir.ActivationFunctionType
    if f == F:
        sched = list(TILE_SCHEDULE)
    else:
        sched = [f] * (FD_TOTAL // f)
    assert sum(sched) == FD_TOTAL, sched
    nt = len(sched)
    fmax = max(sched)

    nc = bacc.Bacc("TRN2", target_bir_lowering=False, debug=False)
    x_in = nc.declare_dram_parameter("x", [NP * 2], f32, isOutput=False)
    t_len = NP * 2 if t_is_i64 else NP
    t_in = nc.declare_dram_parameter("t", [t_len], i32, isOutput=False)
    accA_out = nc.declare_dram_parameter("accA", [P, nt], f32, isOutput=True)
    accB_out = nc.declare_dram_parameter("accB", [P, nt], f32, isOutput=True)

    # (flat_offset, fk) per tile; each tile is a contiguous [P, fk] block
    offs = []
    off = 0
    for fk in sched:
        offs.append((off, fk))
        off += P * fk

    with TileContext(nc) as tc:
        with (
            tc.tile_pool(name="io", bufs=io_bufs) as io,
            tc.tile_pool(name="mid", bufs=mid_bufs) as mid,
            tc.tile_pool(name="accp", bufs=1) as accp,
        ):
            accA = accp.tile([P, nt], f32)
            accB = accp.tile([P, nt], f32)
            if mode in ("dma", "dmasplit"):
                nc.vector.memset(accA[:], 0.0)
                nc.vector.memset(accB[:], 0.0)
            t_dma = nc.sync if t_dma_engine == "sync" else nc.scalar
            for _r in range(reps):
                if rep_barrier:
                    tc.strict_bb_all_engine_barrier()
                for i, (off, fk) in enumerate(offs):
                    x_ap = x_in[off : off + P * fk].rearrange("(p f) -> p f", f=fk)
                    if t_is_i64:
                        t_src = t_in[off : off + P * fk].rearrange(
                            "(p f) -> p f", f=fk)
                        tfk = fk
                    else:
                        t_src = t_in[off // 2 : off // 2 + P * (fk // 2)].rearrange(
                            "(p f) -> p f", f=fk // 2)
                        tfk = fk // 2
                    if t_dma_engine == "alt":
                        x_eng = nc.sync if i % 2 == 0 else nc.scalar
                        t_eng = nc.scalar if i % 2 == 0 else nc.sync
                    else:
                        x_eng, t_eng = nc.sync, t_dma
                    xt = io.tile([P, fk], f32, tag="x")
                    tt = io.tile([P, tfk], i32, tag="t")
                    if mode == "dmasplit":
                        # each stream's tile halves issued on both HWDGE rings
                        h, th = fk // 2, tfk // 2
                        nc.sync.dma_start(out=xt[:, :h], in_=x_ap[:, :h])
                        nc.scalar.dma_start(out=xt[:, h:], in_=x_ap[:, h:])
                        nc.sync.dma_start(out=tt[:, :th], in_=t_src[:, :th])
                        nc.scalar.dma_start(out=tt[:, th:], in_=t_src[:, th:])
                        continue
                    x_eng.dma_start(out=xt[:], in_=x_ap)
                    t_eng.dma_start(out=tt[:], in_=t_src)
                    if mode == "dma":
                        continue
                    t_ap = tt[:, 0::2] if t_is_i64 else tt[:]

                    # w = (z1 + (D0-D1)/2) - z0
                    x0 = mid.tile([P, fk // 2], f32, tag="x0")
                    nc.vector.scalar_tensor_tensor(
                        out=x0[:], in0=xt[:, 1::2], scalar=float((D0 - D1) / 2.0),
                        in1=xt[:, 0::2], op0=Alu.add, op1=Alu.subtract,
                    )
                    # termB row-sums: sum_f t*(w - (D0+D1)/2)
                    jb = mid.tile([P, fk // 2], f32, tag="jb")
                    nc.vector.scalar_tensor_tensor(
                        out=jb[:], in0=x0[:], scalar=float(-(D0 + D1) / 2.0),
                        in1=t_ap, op0=Alu.add, op1=Alu.mult,
                        accum_out=accB[:, i : i + 1],
                    )
                    # termA row-sums: sum_f ln(exp(w) + 1)
                    u = mid.tile([P, fk // 2], f32, tag="u")
                    nc.scalar.activation(out=u[:], in_=x0[:], func=Act.Exp)
                    ja = mid.tile([P, fk // 2], f32, tag="ja")
                    nc.scalar.activation(
                        out=ja[:], in_=u[:], func=Act.Ln, bias=1.0, scale=1.0,
                        accum_out=accA[:, i : i + 1],
                    )
            # accB (last written by DVE) goes out on the SP ring while the
            # final Ln still runs; accA follows on the ACT ring.
            nc.sync.dma_start(out=accB_out[:], in_=accB[:])
            nc.scalar.dma_start(out=accA_out[:], in_=accA[:])
    nc.compile()
    return nc


def _get_program(t_is_i64: bool):
    key = (t_is_i64, 1)
    if key not in _programs:
        _programs[key] = _build(t_is_i64)
    return _programs[key]


def _shard_inputs(output, target):
    output = np.asarray(output)
    target = np.asarray(target)
    assert output.shape == (N, 2), output.shape
    xflat = np.ascontiguousarray(output, dtype=np.float32).reshape(-1)  # [2N]
    if target.dtype == np.int64:
        t_is_i64 = True
        t32 = np.ascontiguousarray(target.reshape(-1)).view(np.int32)  # [2N]
        per_core = NP * 2
    else:
        t_is_i64 = False
        t32 = np.ascontiguousarray(target.reshape(-1), dtype=np.int32)  # [N]
        per_core = NP
    in_maps = [
        {
            "x": xflat[c * NP * 2 : (c + 1) * NP * 2],
            "t": t32[c * per_core : (c + 1) * per_core],
        }
        for c in range(N_CORES)
    ]
    return t_is_i64, in_maps


def kernel(output, target):
    global LAST
    t_is_i64, in_maps = _shard_inputs(output, target)
    nc = _get_program(t_is_i64)
    try:
        LAST = run_bass_kernel_spmd(
            nc, in_maps, core_ids=list(range(N_CORES)), trace=TRACE
        )
    except ModuleNotFoundError:
        # axon NTFF hook unavailable in this environment: run untraced
        LAST = run_bass_kernel_spmd(
            nc, in_maps, core_ids=list(range(N_CORES)), trace=False
        )
    total = np.float64(0.0)
    for r in LAST.results:
        total += r["accA"].astype(np.float64).sum()
        total -= r["accB"].astype(np.float64).sum()
    return np.float32(total)

